# revision 7
# baseline (speedup 1.0000x reference)
"""Sharded cross-attention kernel for 8 TRN2 NeuronCores (Bass/Tile).

Problem: B=4, T=2048, C=1024, H=16 cross-attention
  out = softmax((q Wq + bq)(k Wk + bk)^T / sqrt(64)) (v Wv + bv) Wo + bo

Sharding (communication-free): core c -> batch b = c//2, query-row block
jb = c%2 (1024 of 2048 rows). Each core recomputes its batch's K/V
projections and produces out[b, jb*1024:(jb+1)*1024, :].

v2 vs baseline:
  - all matmul operands bf16 (host-cast): halves DMA traffic and
    LDWEIGHTS cost; PSUM accumulation stays fp32.
  - merged K/V projection loop with early, chunked weight DMAs so the
    first matmul starts ~5us in instead of ~30us.
  - per-head K=64 score matmuls (partition-offset operands) instead of
    the masked pair-packed form: kills the per-p mask generation on the
    vector critical path.
  - av matmuls software-pipelined one t-tile behind the scores so the
    exp latency never blocks the tensor queue head.
  - softmax normalization (reciprocal_approx_fast + partition_broadcast
    + multiply) deferred one pipeline stage off the tensor critical
    path (was: 3.3us full-precision reciprocal at every head boundary).
  - output projection pieces of pair p run as fillers inside pair p+1's
    t-loop, carried across the j boundary; output DMA per 128-row chunk.
"""
import numpy as np
from contextlib import ExitStack

import ml_dtypes

import concourse.tile as tile
from concourse import bacc, mybir
from concourse import bass2jax

B, T, C_FULL = 4, 2048, 1024
N_CORES = 8
_NC_CACHE = {}

F32 = mybir.dt.float32
BF16 = mybir.dt.bfloat16
EXP = mybir.ActivationFunctionType.Exp

KC = 8          # contraction tiles (C = KC*128)
NP = 8          # head pairs
NTK = 16        # key-token tiles (TK = NTK*128)
TQ = 512        # query rows per j-block
NH = 2          # j-blocks per core
TR = 512        # token-range granularity for kT/vT streaming
N_TR = 4        # NTK*128 / TR
C = 1024
TK = 2048
N_TT = TQ // 128   # 4  output row tiles per j
N_EC = C // 512    # 2  output col chunks
W65 = 65           # head dim + ones row


def build_nc(n_cores=8):
    nc = bacc.Bacc("TRN2", target_bir_lowering=False, debug=False,
                   num_devices=n_cores)

    # host-pretiled bf16 inputs (partition dim second-from... [*, 128, ...])
    qT_d = nc.dram_tensor("qT_t", [NH, 128, KC, TQ], BF16, kind="ExternalInput").ap()
    kT_d = nc.dram_tensor("kT_t", [N_TR, 128, KC, TR], BF16, kind="ExternalInput").ap()
    vT_d = nc.dram_tensor("vT_t", [N_TR, 128, KC, TR], BF16, kind="ExternalInput").ap()
    wq_d = nc.dram_tensor("wq_t", [NP, 128, KC, 128], BF16, kind="ExternalInput").ap()
    wk_d = nc.dram_tensor("wk_t", [NP, 128, KC, 128], BF16, kind="ExternalInput").ap()
    wv_d = nc.dram_tensor("wv_t", [128, KC, C], BF16, kind="ExternalInput").ap()
    wo_d = nc.dram_tensor("wo_t", [NP, 128, C], BF16, kind="ExternalInput").ap()
    bq_d = nc.dram_tensor("bq2", [C, 1], F32, kind="ExternalInput").ap()
    bk_d = nc.dram_tensor("bk2", [C, 1], F32, kind="ExternalInput").ap()
    bv_d = nc.dram_tensor("bv2", [1, C], F32, kind="ExternalInput").ap()
    bo_d = nc.dram_tensor("bo2", [1, C], F32, kind="ExternalInput").ap()
    out_d = nc.dram_tensor("out", [NH * TQ, C], F32, kind="ExternalOutput").ap()

    with tile.TileContext(nc) as tc, ExitStack() as top:
        persist = top.enter_context(tc.tile_pool(name="persist", bufs=1))
        khT_sb = persist.tile([128, NP, TK], BF16)      # pair-packed kh^T
        vh_sb = persist.tile([128, NTK, W65 * 2 * NP], BF16)
        vh_grid = vh_sb.rearrange("p t (h c) -> p t h c", c=W65)
        nc.vector.memset(vh_grid[:, :, :, 64], 1.0)

        # ---- biases (small, load once) ----
        bias_pool = top.enter_context(tc.tile_pool(name="bias", bufs=1))
        bk_sb = bias_pool.tile([128, NP], F32)
        nc.sync.dma_start(
            out=bk_sb[:],
            in_=bk_d.rearrange("(np p) one -> p np one", p=128)[:, :, 0])
        bq_sb = bias_pool.tile([128, NP], F32)
        nc.sync.dma_start(
            out=bq_sb[:],
            in_=bq_d.rearrange("(np p) one -> p np one", p=128)[:, :, 0])
        bv_row = bias_pool.tile([1, C], F32)
        nc.sync.dma_start(out=bv_row[:], in_=bv_d[:])
        bv_rep = bias_pool.tile([128, C], F32)
        nc.gpsimd.partition_broadcast(bv_rep[:], bv_row[0:1, :])
        bv_grid = bv_rep.rearrange("p (h c) -> p h c", c=64)
        bo_row = bias_pool.tile([1, C], F32)
        nc.sync.dma_start(out=bo_row[:], in_=bo_d[:])
        bo_rep = bias_pool.tile([128, C], F32)
        nc.gpsimd.partition_broadcast(bo_rep[:], bo_row[0:1, :])

        # ---- persistent-ish weights / qT (DMA'd early, used later) ----
        wq_pool = top.enter_context(tc.tile_pool(name="wq", bufs=1))
        qt_pool = top.enter_context(tc.tile_pool(name="qt", bufs=1))

        # ---- merged K/V projection phase ----
        kv_stack = ExitStack()
        wk_pool = kv_stack.enter_context(tc.tile_pool(name="wk", bufs=1))
        wv_pool = kv_stack.enter_context(tc.tile_pool(name="wv", bufs=1))
        st_pool = kv_stack.enter_context(tc.tile_pool(name="st", bufs=3))
        kvps_pool = kv_stack.enter_context(
            tc.tile_pool(name="kvps", bufs=4, space="PSUM"))

        # weight DMAs, chunked so the first matmul can start after chunk 0
        wk_sb = wk_pool.tile([128, KC, NP, 128], BF16)
        for p in range(NP):
            nc.sync.dma_start(out=wk_sb[:, :, p, :], in_=wk_d[p])
        wv_sb = wv_pool.tile([128, KC, C], BF16)
        for e in range(N_EC):
            nc.sync.dma_start(out=wv_sb[:, :, 512 * e:512 * (e + 1)],
                              in_=wv_d[:, :, 512 * e:512 * (e + 1)])
        wq_sb = wq_pool.tile([128, KC, NP, 128], BF16)
        for p in range(NP):
            nc.sync.dma_start(out=wq_sb[:, :, p, :], in_=wq_d[p])
        qt_sb = qt_pool.tile([128, NH, KC, TQ], BF16)
        for j in range(NH):
            nc.sync.dma_start(out=qt_sb[:, j], in_=qT_d[j])

        for r in range(N_TR):
            kt_t = st_pool.tile([128, KC, TR], BF16, tag="st")
            nc.scalar.dma_start(out=kt_t[:], in_=kT_d[r])
            for p in range(NP):
                ps = kvps_pool.tile([128, TR], F32, tag="kvps")
                for c in range(KC):
                    nc.tensor.matmul(
                        ps[:], wk_sb[:, c, p, :], kt_t[:, c, :],
                        start=(c == 0), stop=(c == KC - 1))
                nc.vector.tensor_scalar_add(
                    khT_sb[:, p, TR * r:TR * (r + 1)], ps[:],
                    bk_sb[:, p:p + 1])
            vt_t = st_pool.tile([128, KC, TR], BF16, tag="st")
            nc.scalar.dma_start(out=vt_t[:], in_=vT_d[r])
            for ti in range(TR // 128):
                t = (TR * r) // 128 + ti
                for e in range(N_EC):
                    ps = kvps_pool.tile([128, 512], F32, tag="kvps")
                    for c in range(KC):
                        nc.tensor.matmul(
                            ps[:], vt_t[:, c, 128 * ti:128 * (ti + 1)],
                            wv_sb[:, c, 512 * e:512 * (e + 1)],
                            start=(c == 0), stop=(c == KC - 1))
                    nc.vector.tensor_add(
                        vh_grid[:, t, 8 * e:8 * (e + 1), 0:64],
                        ps[:].rearrange("p (h c) -> p h c", c=64),
                        bv_grid[:, 8 * e:8 * (e + 1), :])

        kv_stack.close()

        # ---- attention + pipelined output projection ----
        with ExitStack() as ph:
            wo_pool = ph.enter_context(tc.tile_pool(name="wo", bufs=2))
            out_pool = ph.enter_context(tc.tile_pool(name="outp", bufs=2))
            qh_pool = ph.enter_context(tc.tile_pool(name="qh", bufs=2))
            pt_pool = ph.enter_context(tc.tile_pool(name="pt", bufs=4))
            yt_pool = ph.enter_context(tc.tile_pool(name="yt", bufs=2))
            lr_pool = ph.enter_context(tc.tile_pool(name="lr", bufs=4))
            rep_pool = ph.enter_context(tc.tile_pool(name="rep", bufs=4))
            sps_pool = ph.enter_context(
                tc.tile_pool(name="sps", bufs=2, space="PSUM"))
            yps_pool = ph.enter_context(
                tc.tile_pool(name="yps", bufs=4, space="PSUM"))
            qps_pool = ph.enter_context(
                tc.tile_pool(name="qps", bufs=1, space="PSUM"))
            ops_pool = ph.enter_context(
                tc.tile_pool(name="ops", bufs=1, space="PSUM"))

            def qproj(j, p):
                """Q projection matmul chain for pair p of block j -> psum."""
                qps = qps_pool.tile([128, TQ], F32, tag="qps")
                for c in range(KC):
                    nc.tensor.matmul(qps[:], wq_sb[:, c, p, :],
                                     qt_sb[:, j, c, :],
                                     start=(c == 0), stop=(c == KC - 1))
                return qps

            def qevac(qps, p):
                qh = qh_pool.tile([128, TQ], BF16, tag="qh")
                nc.vector.tensor_scalar_add(qh[:], qps[:], bq_sb[:, p:p + 1])
                return qh

            # (pair, yt tile, wo tile, out_sb, j) awaiting output projection
            pending = None

            def opiece(idx):
                p, yt, wo_t, out_sb, _ = pending
                tt, e = divmod(idx, N_EC)
                ops = ops_pool.tile([128, 512], F32, tag="ops")
                nc.tensor.matmul(ops[:], yt[:, 128 * tt:128 * (tt + 1)],
                                 wo_t[:, 512 * e:512 * (e + 1)],
                                 start=True, stop=True)
                if p == 0:
                    nc.vector.tensor_add(
                        out_sb[:, tt, 512 * e:512 * (e + 1)], ops[:],
                        bo_rep[:, 512 * e:512 * (e + 1)])
                else:
                    nc.vector.tensor_add(
                        out_sb[:, tt, 512 * e:512 * (e + 1)],
                        out_sb[:, tt, 512 * e:512 * (e + 1)], ops[:])
                if p == NP - 1 and e == N_EC - 1:
                    _, _, _, osb, oj = pending
                    r0 = TQ * oj + 128 * tt
                    nc.sync.dma_start(out=out_d[r0:r0 + 128, :],
                                      in_=osb[:, tt, :])

            # prologue: pair 0 of j=0
            qh_cur = qevac(qproj(0, 0), 0)

            for j in range(NH):
                out_sb = out_pool.tile([128, N_TT, C], F32, tag="out_sb")
                for p in range(NP):
                    wo_t = wo_pool.tile([128, C], BF16, tag="wo")
                    nc.sync.dma_start(out=wo_t[:], in_=wo_d[p])
                    yps0 = yps_pool.tile([W65, TQ], F32, tag="yps")
                    yps1 = yps_pool.tile([W65, TQ], F32, tag="yps")
                    # filler slots: Q proj for the next pair (possibly next
                    # j) in t=0..7, outproj pieces of the previous pair in
                    # t=8..15
                    nj, np_ = (j, p + 1) if p + 1 < NP else (j + 1, 0)
                    have_q = nj < NH
                    qps_n = None
                    pts = {}
                    for t in range(NTK):
                        sp0 = sps_pool.tile([128, TQ], F32, tag="sps")
                        nc.tensor.matmul(sp0[:],
                                         khT_sb[0:64, p, 128 * t:128 * (t + 1)],
                                         qh_cur[0:64, :],
                                         start=True, stop=True)
                        sp1 = sps_pool.tile([128, TQ], F32, tag="sps")
                        nc.tensor.matmul(sp1[:],
                                         khT_sb[64:128, p, 128 * t:128 * (t + 1)],
                                         qh_cur[64:128, :],
                                         start=True, stop=True)
                        pt0 = pt_pool.tile([128, TQ], BF16, tag="pt")
                        nc.scalar.activation(out=pt0[:], in_=sp0[:],
                                             func=EXP, scale=0.125)
                        pt1 = pt_pool.tile([128, TQ], BF16, tag="pt")
                        nc.scalar.activation(out=pt1[:], in_=sp1[:],
                                             func=EXP, scale=0.125)
                        pts[t] = (pt0, pt1)
                        # av for t-1 (one tile behind: exp latency hidden)
                        if t > 0:
                            q0, q1 = pts.pop(t - 1)
                            nc.tensor.matmul(
                                yps0[:], vh_grid[:, t - 1, 2 * p, :], q0[:],
                                start=(t - 1 == 0), stop=False)
                            nc.tensor.matmul(
                                yps1[:], vh_grid[:, t - 1, 2 * p + 1, :], q1[:],
                                start=(t - 1 == 0), stop=False)
                        # one filler per t-slot
                        if t < KC:
                            if have_q:
                                if qps_n is None:
                                    qps_n = qps_pool.tile([128, TQ], F32,
                                                          tag="qps")
                                nc.tensor.matmul(
                                    qps_n[:], wq_sb[:, t, np_, :],
                                    qt_sb[:, nj, t, :],
                                    start=(t == 0), stop=(t == KC - 1))
                        else:
                            if pending is not None:
                                opiece(t - KC)
                    # tail: av for t=15, close accumulation
                    q0, q1 = pts.pop(NTK - 1)
                    nc.tensor.matmul(yps0[:], vh_grid[:, NTK - 1, 2 * p, :],
                                     q0[:], start=False, stop=True)
                    nc.tensor.matmul(yps1[:], vh_grid[:, NTK - 1, 2 * p + 1, :],
                                     q1[:], start=False, stop=True)
                    if have_q:
                        qh_cur = qevac(qps_n, np_)
                    # deferred normalization for this pair: executes on
                    # vector/gpsimd while pair p+1's t-loop runs on tensor.
                    # One exact reciprocal covers both heads (DVE lanes are
                    # per-partition, so [2,512] costs the same as [1,512]).
                    yt = yt_pool.tile([128, TQ], BF16, tag="yt")
                    for s, yps in ((0, yps0), (1, yps1)):
                        l0 = lr_pool.tile([1, TQ], F32, tag="l0")
                        nc.vector.reciprocal(l0[:], yps[64:65, :])
                        rep = rep_pool.tile([64, TQ], F32, tag="rep")
                        nc.gpsimd.partition_broadcast(rep[:], l0[0:1, :])
                        nc.vector.tensor_mul(yt[64 * s:64 * (s + 1), :],
                                             yps[0:64, :], rep[:])
                    pending = (p, yt, wo_t, out_sb, j)
            # epilogue: last pair's output projection
            for idx in range(N_TT * N_EC):
                opiece(idx)

    nc.compile()
    return nc


def _marshal(q, k, v, Wq, bq, Wk, bk, Wv, bv, Wo, bo):
    bf = ml_dtypes.bfloat16
    f32 = np.float32

    def cast(x):
        return np.ascontiguousarray(np.asarray(x, f32).astype(bf))

    wk_t = cast(np.asarray(Wk, f32).reshape(KC, 128, NP, 128)
                .transpose(2, 1, 0, 3))
    wq_t = cast(np.asarray(Wq, f32).reshape(KC, 128, NP, 128)
                .transpose(2, 1, 0, 3))
    wv_t = cast(np.asarray(Wv, f32).reshape(KC, 128, C).transpose(1, 0, 2))
    wo_t = cast(np.asarray(Wo, f32).reshape(NP, 128, C))
    shared = {
        "wq_t": wq_t, "wk_t": wk_t, "wv_t": wv_t, "wo_t": wo_t,
        "bq2": np.ascontiguousarray(np.asarray(bq, f32).reshape(C, 1)),
        "bk2": np.ascontiguousarray(np.asarray(bk, f32).reshape(C, 1)),
        "bv2": np.ascontiguousarray(np.asarray(bv, f32).reshape(1, C)),
        "bo2": np.ascontiguousarray(np.asarray(bo, f32).reshape(1, C)),
    }
    kT = {}
    vT = {}
    for b in range(B):
        kT[b] = cast(k[b].T.reshape(KC, 128, N_TR, TR).transpose(2, 1, 0, 3))
        vT[b] = cast(v[b].T.reshape(KC, 128, N_TR, TR).transpose(2, 1, 0, 3))
    in_maps = []
    for c in range(N_CORES):
        b, jb = divmod(c, 2)
        im = dict(shared)
        qc = q[b, 1024 * jb:1024 * (jb + 1)]
        im["qT_t"] = cast(qc.T.reshape(KC, 128, NH, TQ).transpose(2, 1, 0, 3))
        im["kT_t"] = kT[b]
        im["vT_t"] = vT[b]
        in_maps.append(im)
    return in_maps


def kernel(q, k, v, Wq, bq, Wk, bk, Wv, bv, Wo, bo):
    q = np.asarray(q, np.float32)
    k = np.asarray(k, np.float32)
    v = np.asarray(v, np.float32)
    if "nc" not in _NC_CACHE:
        _NC_CACHE["nc"] = build_nc()
    nc = _NC_CACHE["nc"]
    in_maps = _marshal(q, k, v, Wq, bq, Wk, bk, Wv, bv, Wo, bo)
    results = bass2jax.run_bass_via_pjrt(nc, in_maps, n_cores=N_CORES)
    out = np.zeros((B, T, C_FULL), np.float32)
    for c in range(N_CORES):
        b, jb = divmod(c, 2)
        out[b, 1024 * jb:1024 * (jb + 1)] = results[c]["out"]
    return out


# revision 9
# speedup vs baseline: 1.0210x; 1.0210x over previous
"""Sharded cross-attention kernel for 8 TRN2 NeuronCores (Bass/Tile).

Problem: B=4, T=2048, C=1024, H=16 cross-attention
  out = softmax((q Wq + bq)(k Wk + bk)^T / sqrt(64)) (v Wv + bv) Wo + bo

Sharding (communication-free): core c -> batch b = c//2, query-row block
jb = c%2 (1024 of 2048 rows). Each core recomputes its batch's K/V
projections and produces out[b, jb*1024:(jb+1)*1024, :].

v2 vs baseline:
  - all matmul operands bf16 (host-cast): halves DMA traffic and
    LDWEIGHTS cost; PSUM accumulation stays fp32.
  - merged K/V projection loop with early, chunked weight DMAs so the
    first matmul starts ~5us in instead of ~30us.
  - per-head K=64 score matmuls (partition-offset operands) instead of
    the masked pair-packed form: kills the per-p mask generation on the
    vector critical path.
  - av matmuls software-pipelined one t-tile behind the scores so the
    exp latency never blocks the tensor queue head.
  - softmax normalization (reciprocal_approx_fast + partition_broadcast
    + multiply) deferred one pipeline stage off the tensor critical
    path (was: 3.3us full-precision reciprocal at every head boundary).
  - output projection pieces of pair p run as fillers inside pair p+1's
    t-loop, carried across the j boundary; output DMA per 128-row chunk.
"""
import numpy as np
from contextlib import ExitStack

import ml_dtypes

import concourse.tile as tile
from concourse import bacc, mybir
from concourse import bass2jax

B, T, C_FULL = 4, 2048, 1024
N_CORES = 8
_NC_CACHE = {}

F32 = mybir.dt.float32
BF16 = mybir.dt.bfloat16
EXP = mybir.ActivationFunctionType.Exp

KC = 8          # contraction tiles (C = KC*128)
NP = 8          # head pairs
NTK = 16        # key-token tiles (TK = NTK*128)
TQ = 512        # query rows per j-block
NH = 2          # j-blocks per core
TR = 512        # token-range granularity for kT/vT streaming
N_TR = 4        # NTK*128 / TR
C = 1024
TK = 2048
N_TT = TQ // 128   # 4  output row tiles per j
N_EC = C // 512    # 2  output col chunks
W65 = 65           # head dim + ones row


def build_nc(n_cores=8):
    nc = bacc.Bacc("TRN2", target_bir_lowering=False, debug=False,
                   num_devices=n_cores)

    # host-pretiled bf16 inputs (partition dim second-from... [*, 128, ...])
    qT_d = nc.dram_tensor("qT_t", [NH, 128, KC, TQ], BF16, kind="ExternalInput").ap()
    kT_d = nc.dram_tensor("kT_t", [N_TR, 128, KC, TR], BF16, kind="ExternalInput").ap()
    vT_d = nc.dram_tensor("vT_t", [N_TR, 128, KC, TR], BF16, kind="ExternalInput").ap()
    wq_d = nc.dram_tensor("wq_t", [NP, 128, KC, 128], BF16, kind="ExternalInput").ap()
    wk_d = nc.dram_tensor("wk_t", [NP, 128, KC, 128], BF16, kind="ExternalInput").ap()
    wv_d = nc.dram_tensor("wv_t", [128, KC, C], BF16, kind="ExternalInput").ap()
    wo_d = nc.dram_tensor("wo_t", [NP, 128, C], BF16, kind="ExternalInput").ap()
    bq_d = nc.dram_tensor("bq2", [C, 1], F32, kind="ExternalInput").ap()
    bk_d = nc.dram_tensor("bk2", [C, 1], F32, kind="ExternalInput").ap()
    bv_d = nc.dram_tensor("bv2", [1, C], F32, kind="ExternalInput").ap()
    bo_d = nc.dram_tensor("bo2", [1, C], F32, kind="ExternalInput").ap()
    out_d = nc.dram_tensor("out", [NH * TQ, C], F32, kind="ExternalOutput").ap()

    with tile.TileContext(nc) as tc, ExitStack() as top:
        persist = top.enter_context(tc.tile_pool(name="persist", bufs=1))
        khT_sb = persist.tile([128, NP, TK], BF16)      # pair-packed kh^T
        vh_sb = persist.tile([128, NTK, W65 * 2 * NP], BF16)
        vh_grid = vh_sb.rearrange("p t (h c) -> p t h c", c=W65)
        nc.vector.memset(vh_grid[:, :, :, 64], 1.0)

        # ---- biases (small, load once) ----
        bias_pool = top.enter_context(tc.tile_pool(name="bias", bufs=1))
        bk_sb = bias_pool.tile([128, NP], F32)
        nc.sync.dma_start(
            out=bk_sb[:],
            in_=bk_d.rearrange("(np p) one -> p np one", p=128)[:, :, 0])
        bq_sb = bias_pool.tile([128, NP], F32)
        nc.sync.dma_start(
            out=bq_sb[:],
            in_=bq_d.rearrange("(np p) one -> p np one", p=128)[:, :, 0])
        bv_row = bias_pool.tile([1, C], F32)
        nc.sync.dma_start(out=bv_row[:], in_=bv_d[:])
        bv_rep = bias_pool.tile([128, C], F32)
        nc.gpsimd.partition_broadcast(bv_rep[:], bv_row[0:1, :])
        bv_grid = bv_rep.rearrange("p (h c) -> p h c", c=64)
        bo_row = bias_pool.tile([1, C], F32)
        nc.sync.dma_start(out=bo_row[:], in_=bo_d[:])
        bo_rep = bias_pool.tile([128, C], F32)
        nc.gpsimd.partition_broadcast(bo_rep[:], bo_row[0:1, :])

        # ---- persistent-ish weights / qT (DMA'd early, used later) ----
        wq_pool = top.enter_context(tc.tile_pool(name="wq", bufs=1))
        qt_pool = top.enter_context(tc.tile_pool(name="qt", bufs=1))

        # ---- merged K/V projection phase ----
        kv_stack = ExitStack()
        wk_pool = kv_stack.enter_context(tc.tile_pool(name="wk", bufs=1))
        wv_pool = kv_stack.enter_context(tc.tile_pool(name="wv", bufs=1))
        st_pool = kv_stack.enter_context(tc.tile_pool(name="st", bufs=3))
        kvps_pool = kv_stack.enter_context(
            tc.tile_pool(name="kvps", bufs=4, space="PSUM"))

        # weight DMAs, chunked so the first matmul can start after chunk 0
        wk_sb = wk_pool.tile([128, KC, NP, 128], BF16)
        for p in range(NP):
            nc.sync.dma_start(out=wk_sb[:, :, p, :], in_=wk_d[p])
        wv_sb = wv_pool.tile([128, KC, C], BF16)
        for e in range(N_EC):
            nc.sync.dma_start(out=wv_sb[:, :, 512 * e:512 * (e + 1)],
                              in_=wv_d[:, :, 512 * e:512 * (e + 1)])
        wq_sb = wq_pool.tile([128, KC, NP, 128], BF16)
        for p in range(NP):
            nc.sync.dma_start(out=wq_sb[:, :, p, :], in_=wq_d[p])
        qt_sb = qt_pool.tile([128, NH, KC, TQ], BF16)
        for j in range(NH):
            nc.sync.dma_start(out=qt_sb[:, j], in_=qT_d[j])

        for r in range(N_TR):
            kt_t = st_pool.tile([128, KC, TR], BF16, tag="st")
            nc.scalar.dma_start(out=kt_t[:], in_=kT_d[r])
            for p in range(NP):
                ps = kvps_pool.tile([128, TR], F32, tag="kvps")
                for c in range(KC):
                    nc.tensor.matmul(
                        ps[:], wk_sb[:, c, p, :], kt_t[:, c, :],
                        start=(c == 0), stop=(c == KC - 1))
                nc.vector.tensor_scalar_add(
                    khT_sb[:, p, TR * r:TR * (r + 1)], ps[:],
                    bk_sb[:, p:p + 1])
            vt_t = st_pool.tile([128, KC, TR], BF16, tag="st")
            nc.scalar.dma_start(out=vt_t[:], in_=vT_d[r])
            for ti in range(TR // 128):
                t = (TR * r) // 128 + ti
                for e in range(N_EC):
                    ps = kvps_pool.tile([128, 512], F32, tag="kvps")
                    for c in range(KC):
                        nc.tensor.matmul(
                            ps[:], vt_t[:, c, 128 * ti:128 * (ti + 1)],
                            wv_sb[:, c, 512 * e:512 * (e + 1)],
                            start=(c == 0), stop=(c == KC - 1))
                    nc.vector.tensor_add(
                        vh_grid[:, t, 8 * e:8 * (e + 1), 0:64],
                        ps[:].rearrange("p (h c) -> p h c", c=64),
                        bv_grid[:, 8 * e:8 * (e + 1), :])

        kv_stack.close()

        # ---- attention + pipelined output projection ----
        with ExitStack() as ph:
            wo_pool = ph.enter_context(tc.tile_pool(name="wo", bufs=2))
            out_pool = ph.enter_context(tc.tile_pool(name="outp", bufs=2))
            qh_pool = ph.enter_context(tc.tile_pool(name="qh", bufs=2))
            pt_pool = ph.enter_context(tc.tile_pool(name="pt", bufs=4))
            yt_pool = ph.enter_context(tc.tile_pool(name="yt", bufs=2))
            lr_pool = ph.enter_context(tc.tile_pool(name="lr", bufs=4))
            rep_pool = ph.enter_context(tc.tile_pool(name="rep", bufs=4))
            sps_pool = ph.enter_context(
                tc.tile_pool(name="sps", bufs=3, space="PSUM"))
            yps_pool = ph.enter_context(
                tc.tile_pool(name="yps", bufs=3, space="PSUM"))
            qps_pool = ph.enter_context(
                tc.tile_pool(name="qps", bufs=1, space="PSUM"))
            ops_pool = ph.enter_context(
                tc.tile_pool(name="ops", bufs=1, space="PSUM"))

            def qproj(j, p):
                """Q projection matmul chain for pair p of block j -> psum."""
                qps = qps_pool.tile([128, TQ], F32, tag="qps")
                for c in range(KC):
                    nc.tensor.matmul(qps[:], wq_sb[:, c, p, :],
                                     qt_sb[:, j, c, :],
                                     start=(c == 0), stop=(c == KC - 1))
                return qps

            def qevac(qps, p):
                qh = qh_pool.tile([128, TQ], BF16, tag="qh")
                nc.vector.tensor_scalar_add(qh[:], qps[:], bq_sb[:, p:p + 1])
                return qh

            # (pair, yt tile, wo tile, out_sb, j) awaiting output projection
            pending = None

            def opiece(idx):
                p, yt, wo_t, out_sb, _ = pending
                tt, e = divmod(idx, N_EC)
                ops = ops_pool.tile([128, 512], F32, tag="ops")
                nc.tensor.matmul(ops[:], yt[:, 128 * tt:128 * (tt + 1)],
                                 wo_t[:, 512 * e:512 * (e + 1)],
                                 start=True, stop=True)
                if p == 0:
                    nc.vector.tensor_add(
                        out_sb[:, tt, 512 * e:512 * (e + 1)], ops[:],
                        bo_rep[:, 512 * e:512 * (e + 1)])
                else:
                    nc.vector.tensor_add(
                        out_sb[:, tt, 512 * e:512 * (e + 1)],
                        out_sb[:, tt, 512 * e:512 * (e + 1)], ops[:])
                if p == NP - 1 and e == N_EC - 1:
                    _, _, _, osb, oj = pending
                    r0 = TQ * oj + 128 * tt
                    nc.sync.dma_start(out=out_d[r0:r0 + 128, :],
                                      in_=osb[:, tt, :])

            # prologue: pair 0 of j=0
            qh_cur = qevac(qproj(0, 0), 0)

            for j in range(NH):
                out_sb = out_pool.tile([128, N_TT, C], F32, tag="out_sb")
                for p in range(NP):
                    wo_t = wo_pool.tile([128, C], BF16, tag="wo")
                    nc.sync.dma_start(out=wo_t[:], in_=wo_d[p])
                    nj, np_ = (j, p + 1) if p + 1 < NP else (j + 1, 0)
                    have_q = nj < NH
                    qps_n = None
                    qh_next = None
                    yt = yt_pool.tile([128, TQ], BF16, tag="yt")
                    for s in range(2):
                        # head h = 2p+s: 16 t-tiles of scores -> exp -> av,
                        # with av one tile behind so the exp latency never
                        # blocks the tensor queue head. Fillers in every
                        # other slot: s=0 runs the next pair's Q projection,
                        # s=1 the previous pair's output projection. Each
                        # head's softmax normalization is deferred to run on
                        # vector/gpsimd under the next sub-loop's matmuls.
                        yps = yps_pool.tile([W65, TQ], F32, tag="yps")
                        pts = {}
                        for t in range(NTK):
                            sp = sps_pool.tile([128, TQ], F32, tag="sps")
                            nc.tensor.matmul(
                                sp[:],
                                khT_sb[64 * s:64 * (s + 1), p,
                                       128 * t:128 * (t + 1)],
                                qh_cur[64 * s:64 * (s + 1), :],
                                start=True, stop=True)
                            pt = pt_pool.tile([128, TQ], BF16, tag="pt")
                            nc.scalar.activation(out=pt[:], in_=sp[:],
                                                 func=EXP, scale=0.125)
                            pts[t] = pt
                            if t > 0:
                                nc.tensor.matmul(
                                    yps[:], vh_grid[:, t - 1, 2 * p + s, :],
                                    pts.pop(t - 1)[:],
                                    start=(t - 1 == 0), stop=False)
                            if t % 2 == 0:
                                g = t // 2
                                if s == 0:
                                    if have_q:
                                        if qps_n is None:
                                            qps_n = qps_pool.tile(
                                                [128, TQ], F32, tag="qps")
                                        nc.tensor.matmul(
                                            qps_n[:], wq_sb[:, g, np_, :],
                                            qt_sb[:, nj, g, :],
                                            start=(g == 0), stop=(g == KC - 1))
                                elif pending is not None:
                                    opiece(g)
                        nc.tensor.matmul(
                            yps[:], vh_grid[:, NTK - 1, 2 * p + s, :],
                            pts.pop(NTK - 1)[:], start=False, stop=True)
                        if s == 0 and have_q:
                            qh_next = qevac(qps_n, np_)
                        l0 = lr_pool.tile([1, TQ], F32, tag="l0")
                        nc.vector.reciprocal(l0[:], yps[64:65, :])
                        rep = rep_pool.tile([64, TQ], F32, tag="rep")
                        nc.gpsimd.partition_broadcast(rep[:], l0[0:1, :])
                        nc.vector.tensor_mul(yt[64 * s:64 * (s + 1), :],
                                             yps[0:64, :], rep[:])
                    if qh_next is not None:
                        qh_cur = qh_next
                    pending = (p, yt, wo_t, out_sb, j)
            # epilogue: last pair's output projection
            for idx in range(N_TT * N_EC):
                opiece(idx)

    nc.compile()
    return nc


def _marshal(q, k, v, Wq, bq, Wk, bk, Wv, bv, Wo, bo):
    bf = ml_dtypes.bfloat16
    f32 = np.float32

    def cast(x):
        return np.ascontiguousarray(np.asarray(x, f32).astype(bf))

    wk_t = cast(np.asarray(Wk, f32).reshape(KC, 128, NP, 128)
                .transpose(2, 1, 0, 3))
    wq_t = cast(np.asarray(Wq, f32).reshape(KC, 128, NP, 128)
                .transpose(2, 1, 0, 3))
    wv_t = cast(np.asarray(Wv, f32).reshape(KC, 128, C).transpose(1, 0, 2))
    wo_t = cast(np.asarray(Wo, f32).reshape(NP, 128, C))
    shared = {
        "wq_t": wq_t, "wk_t": wk_t, "wv_t": wv_t, "wo_t": wo_t,
        "bq2": np.ascontiguousarray(np.asarray(bq, f32).reshape(C, 1)),
        "bk2": np.ascontiguousarray(np.asarray(bk, f32).reshape(C, 1)),
        "bv2": np.ascontiguousarray(np.asarray(bv, f32).reshape(1, C)),
        "bo2": np.ascontiguousarray(np.asarray(bo, f32).reshape(1, C)),
    }
    kT = {}
    vT = {}
    for b in range(B):
        kT[b] = cast(k[b].T.reshape(KC, 128, N_TR, TR).transpose(2, 1, 0, 3))
        vT[b] = cast(v[b].T.reshape(KC, 128, N_TR, TR).transpose(2, 1, 0, 3))
    in_maps = []
    for c in range(N_CORES):
        b, jb = divmod(c, 2)
        im = dict(shared)
        qc = q[b, 1024 * jb:1024 * (jb + 1)]
        im["qT_t"] = cast(qc.T.reshape(KC, 128, NH, TQ).transpose(2, 1, 0, 3))
        im["kT_t"] = kT[b]
        im["vT_t"] = vT[b]
        in_maps.append(im)
    return in_maps


def kernel(q, k, v, Wq, bq, Wk, bk, Wv, bv, Wo, bo):
    q = np.asarray(q, np.float32)
    k = np.asarray(k, np.float32)
    v = np.asarray(v, np.float32)
    if "nc" not in _NC_CACHE:
        _NC_CACHE["nc"] = build_nc()
    nc = _NC_CACHE["nc"]
    in_maps = _marshal(q, k, v, Wq, bq, Wk, bk, Wv, bv, Wo, bo)
    results = bass2jax.run_bass_via_pjrt(nc, in_maps, n_cores=N_CORES)
    out = np.zeros((B, T, C_FULL), np.float32)
    for c in range(N_CORES):
        b, jb = divmod(c, 2)
        out[b, 1024 * jb:1024 * (jb + 1)] = results[c]["out"]
    return out
